# revision 1
# baseline (speedup 1.0000x reference)
"""Trainium2 Bass kernel for CustomGRUModel.

Reference computation (per batch row):
    gx = x @ W                       # [T, 3H] input projections (precomputed)
    per step t:
        gh_zr = h @ U[:, :2H]
        z = sigmoid(gxz + ghz + bz)
        r = sigmoid(gxr + ghr + br)
        n = tanh(gxn + (r*h) @ U[:, 2H:] + bn)
        h = z*h + (1-z)*n
    y = h_last @ Wd + bd

Sharding: data-parallel over batch, 32 rows per core on 8 cores. Weights
replicated. No collectives.

Per-core layout: everything transposed ("feature on partitions"):
  hT [H=512, B=32] stored as one SBUF tile [128, 4*32] (4 H-chunks packed in
  the free dim). Recurrent matmuls keep U as the stationary operand
  (lhsT = U k-tile slice [128, 128], fp32 exact) streaming hT chunks (N=32):
  output lands transposed [3H-chunk, B] in PSUM, which makes the gate
  elementwise work run on full 128 partitions.

The gx precompute runs chunked (16 steps at a time) in float32r (1 cyc/row at
N=512), interleaved between recurrence steps so it fills TensorE gaps. x is
transposed on-chip with PE transposes. The bias b is folded into the
PSUM->SBUF eviction of gx (ACT activation bias).
"""

import os

import numpy as np

B, T, D, H = 256, 512, 256, 512
NCORES = 8
BL = B // NCORES  # 32 batch rows per core
TC = 16  # timestep chunk for the gx precompute
KH = H // 128  # 4 k-tiles over H
KD = D // 128  # 2 k-tiles over D
M3H = 3 * H // 128  # 12 m-tiles over 3H

_CACHE = {}


def _build(t_run):
    from contextlib import ExitStack

    import concourse.bacc as bacc
    import concourse.bass as bass
    import concourse.tile as tile
    from concourse import masks, mybir

    dt = mybir.dt
    f32 = dt.float32
    f32r = dt.float32r
    AF = mybir.ActivationFunctionType

    nchunk = t_run // TC

    nc = bacc.Bacc(
        "TRN2", target_bir_lowering=False, debug=False, num_devices=NCORES
    )
    x_d = nc.dram_tensor("x", [BL, T, D], f32, kind="ExternalInput")
    w_d = nc.dram_tensor("W", [D, 3 * H], f32, kind="ExternalInput")
    u_d = nc.dram_tensor("U", [H, 3 * H], f32, kind="ExternalInput")
    b_d = nc.dram_tensor("b", [3 * H], f32, kind="ExternalInput")
    wd_d = nc.dram_tensor("Wd", [H, 1], f32, kind="ExternalInput")
    bd_d = nc.dram_tensor("bd", [1], f32, kind="ExternalInput")
    y_d = nc.dram_tensor("y", [BL, 1], f32, kind="ExternalOutput")

    # chunked view of x: [chunk, tc, b, d]
    x_view = x_d.rearrange("b (c t) d -> c t b d", t=TC)

    with tile.TileContext(nc) as tc, ExitStack() as ctx:
        const = ctx.enter_context(tc.tile_pool(name="const", bufs=1))
        gx_pool = ctx.enter_context(tc.tile_pool(name="gx", bufs=2))
        xin_pool = ctx.enter_context(tc.tile_pool(name="xin", bufs=8))
        xt_pool = ctx.enter_context(tc.tile_pool(name="xt", bufs=2))
        sb_pool = ctx.enter_context(tc.tile_pool(name="sb", bufs=3))
        zr_psum = ctx.enter_context(
            tc.tile_pool(name="zrp", bufs=2, space=bass.MemorySpace.PSUM)
        )
        n_psum = ctx.enter_context(
            tc.tile_pool(name="np", bufs=2, space=bass.MemorySpace.PSUM)
        )
        pre_psum = ctx.enter_context(
            tc.tile_pool(name="prep", bufs=2, space=bass.MemorySpace.PSUM)
        )
        xt_psum = ctx.enter_context(
            tc.tile_pool(name="xtp", bufs=2, space=bass.MemorySpace.PSUM)
        )

        # ---- constants ----
        w_stage = const.tile([128, KD, 3 * H], f32)
        for k in range(KD):
            nc.sync.dma_start(w_stage[:, k, :], w_d[k * 128 : (k + 1) * 128, :])
        w_sb = const.tile([128, KD, 3 * H], f32r)
        for k in range(KD):
            nc.scalar.copy(w_sb[:, k, :], w_stage[:, k, :])
        u_sb = const.tile([128, KH, 3 * H], f32)
        for k in range(KH):
            nc.sync.dma_start(u_sb[:, k, :], u_d[k * 128 : (k + 1) * 128, :])
        b_sb = const.tile([128, M3H], f32)
        nc.sync.dma_start(b_sb[:], b_d.rearrange("(m p) -> p m", p=128))
        wd_sb = const.tile([128, KH], f32)
        nc.sync.dma_start(wd_sb[:], wd_d.rearrange("(k p) o -> p (k o)", p=128))
        bd_sb = const.tile([1, 1], f32)
        nc.sync.dma_start(bd_sb[0:1, :], bd_d.rearrange("(o u) -> o u", u=1))
        ident = const.tile([128, 128], f32)
        masks.make_identity(nc, ident[:])
        ones_sb = const.tile([1, BL], f32)
        nc.gpsimd.memset(ones_sb[0:1, :], 1.0)

        # persistent hidden state hT: [128, (k, b)] = [128, 4*32]
        h_sb = const.tile([128, KH * BL], f32)
        nc.gpsimd.memset(h_sb[:], 0.0)

        warm_ps = n_psum.tile([128, 128], f32, name="warm", tag="np")
        nc.tensor.transpose(warm_ps[:], ident[:], ident[:])

        gx_tiles = {}

        def make_units(c):
            """Emit-thunks for precomputing gx chunk c (16 steps)."""
            gx_t = gx_pool.tile([128, TC, M3H, BL], f32, name="gx", tag="gx")
            gx_tiles[c] = gx_t
            xins = []
            xt_sb = xt_pool.tile([128, KD, TC * BL], f32r, name="xt", tag="xt")
            xt_ps = {}
            units = []

            def load(j):
                t = xin_pool.tile([128, D], f32, name="xin", tag="xin")
                xins.append(t)
                nc.sync.dma_start(
                    t[:],
                    x_view[c, 4 * j : 4 * (j + 1)],
                )

            def tr(j):
                # transpose both d-chunks of xin row-block j
                for kd in range(KD):
                    if j == 0:
                        xt_ps[kd] = xt_psum.tile([128, TC * BL], f32, name="xtp", tag="xtp")
                    nc.tensor.transpose(
                        xt_ps[kd][:, 128 * j : 128 * (j + 1)],
                        xins[j][:, 128 * kd : 128 * (kd + 1)],
                        ident[:],
                    )

            def evict_xt():
                for kd in range(KD):
                    nc.scalar.copy(xt_sb[:, kd, :], xt_ps[kd][:])

            def mm(m):
                ps = pre_psum.tile([128, TC * BL], f32, name="prep", tag="prep")
                for kd in range(KD):
                    nc.tensor.matmul(
                        ps[:],
                        w_sb[:, kd, m * 128 : (m + 1) * 128],
                        xt_sb[:, kd, :],
                        start=(kd == 0),
                        stop=(kd == KD - 1),
                    )
                nc.scalar.activation(
                    gx_t[:, :, m, :],
                    ps[:].rearrange("p (t b) -> p t b", t=TC),
                    AF.Identity,
                    bias=b_sb[:, m : m + 1],
                )

            for j in range(4):
                units.append(lambda j=j: load(j))
            for j in range(4):
                units.append(lambda j=j: tr(j))
            units.append(evict_xt)
            for m in range(M3H):
                units.append(lambda m=m: mm(m))
            return units

        def emit_step(c, j):
            gx_t = gx_tiles[c]
            zr_ps = zr_psum.tile([128, 8 * BL], f32, name="zrp", tag="zrp")
            # r-gate matmuls first (m 4..7), then z (m 0..3), so the
            # r -> rh -> n chain overlaps the z matmuls on PE.
            for m in [4, 5, 6, 7, 0, 1, 2, 3]:
                for k in range(KH):
                    nc.tensor.matmul(
                        zr_ps[:, m * BL : (m + 1) * BL],
                        u_sb[:, k, m * 128 : (m + 1) * 128],
                        h_sb[:, k * BL : (k + 1) * BL],
                        start=(k == 0),
                        stop=(k == KH - 1),
                    )
            gr_sb = sb_pool.tile([128, 4 * BL], f32, name="gr", tag="gr")
            nc.vector.tensor_add(
                gr_sb[:].rearrange("p (m b) -> p m b", m=4),
                zr_ps[:, 4 * BL : 8 * BL].rearrange("p (m b) -> p m b", m=4),
                gx_t[:, j, 4:8, :],
            )
            r_sb = sb_pool.tile([128, 4 * BL], f32, name="r", tag="r")
            nc.scalar.activation(r_sb[:], gr_sb[:], AF.Sigmoid)
            rh_sb = sb_pool.tile([128, 4 * BL], f32, name="rh", tag="rh")
            nc.vector.tensor_mul(rh_sb[:], r_sb[:], h_sb[:])

            n_ps = n_psum.tile([128, 4 * BL], f32, name="npt", tag="np")
            for m in range(4):
                for k in range(KH):
                    nc.tensor.matmul(
                        n_ps[:, m * BL : (m + 1) * BL],
                        u_sb[:, k, 1024 + m * 128 : 1024 + (m + 1) * 128],
                        rh_sb[:, k * BL : (k + 1) * BL],
                        start=(k == 0),
                        stop=(k == KH - 1),
                    )

            gz_sb = sb_pool.tile([128, 4 * BL], f32, name="gz", tag="gz")
            nc.vector.tensor_add(
                gz_sb[:].rearrange("p (m b) -> p m b", m=4),
                zr_ps[:, 0 : 4 * BL].rearrange("p (m b) -> p m b", m=4),
                gx_t[:, j, 0:4, :],
            )
            z_sb = sb_pool.tile([128, 4 * BL], f32, name="z", tag="z")
            nc.scalar.activation(z_sb[:], gz_sb[:], AF.Sigmoid)

            gn_sb = sb_pool.tile([128, 4 * BL], f32, name="gn", tag="gn")
            nc.vector.tensor_add(
                gn_sb[:].rearrange("p (m b) -> p m b", m=4),
                n_ps[:].rearrange("p (m b) -> p m b", m=4),
                gx_t[:, j, 8:12, :],
            )
            n_sb = sb_pool.tile([128, 4 * BL], f32, name="n", tag="n")
            nc.scalar.activation(n_sb[:], gn_sb[:], AF.Tanh)

            # h = n + z*(h - n)
            tmp = sb_pool.tile([128, 4 * BL], f32, name="tmp", tag="tmp")
            nc.vector.tensor_sub(tmp[:], h_sb[:], n_sb[:])
            nc.vector.tensor_mul(tmp[:], z_sb[:], tmp[:])
            nc.vector.tensor_add(h_sb[:], n_sb[:], tmp[:])

        # ---- main emission ----
        # Chunk 0's precompute up front; chunk c+1's precompute interleaved
        # between chunk c's recurrence steps so it fills TensorE gaps.
        for u in make_units(0):
            u()
        for c in range(nchunk):
            pend = make_units(c + 1) if c + 1 < nchunk else []
            done = 0
            for j in range(TC):
                emit_step(c, j)
                want = (len(pend) * (j + 1) + TC - 1) // TC
                while done < min(want, len(pend)):
                    pend[done]()
                    done += 1
            while done < len(pend):
                pend[done]()
                done += 1

        # final dense head: y = h @ Wd + bd
        out_ps = n_psum.tile([BL, 1], f32, name="outp", tag="np")
        for k in range(KH):
            nc.tensor.matmul(
                out_ps[:],
                h_sb[:, k * BL : (k + 1) * BL],
                wd_sb[:, k : k + 1],
                start=(k == 0),
                stop=False,
            )
        nc.tensor.matmul(
            out_ps[:], ones_sb[0:1, :], bd_sb[0:1, :], start=False, stop=True
        )
        y_sb = sb_pool.tile([BL, 1], f32, name="y", tag="y")
        nc.vector.tensor_copy(y_sb[:], out_ps[:])
        nc.sync.dma_start(y_d[:], y_sb[:])

    nc.compile()
    return nc


def kernel(x, W, U, b, Wd, bd):
    from concourse.bass_utils import run_bass_kernel_spmd

    t_run = int(os.environ.get("GRU_T_RUN", T))
    key = t_run
    if key not in _CACHE:
        _CACHE[key] = _build(t_run)
    nc = _CACHE[key]

    x = np.ascontiguousarray(np.asarray(x, dtype=np.float32))
    W = np.ascontiguousarray(np.asarray(W, dtype=np.float32))
    U = np.ascontiguousarray(np.asarray(U, dtype=np.float32))
    b = np.ascontiguousarray(np.asarray(b, dtype=np.float32))
    Wd = np.ascontiguousarray(np.asarray(Wd, dtype=np.float32))
    bd = np.ascontiguousarray(np.asarray(bd, dtype=np.float32))

    in_maps = [
        {
            "x": np.ascontiguousarray(x[i * BL : (i + 1) * BL]),
            "W": W,
            "U": U,
            "b": b,
            "Wd": Wd,
            "bd": bd,
        }
        for i in range(NCORES)
    ]
    res = run_bass_kernel_spmd(
        nc,
        in_maps,
        core_ids=list(range(NCORES)),
        trace=os.environ.get("GRU_TRACE", "0") == "1",
    )
    out = np.concatenate([r["y"] for r in res.results], axis=0)
    if res.exec_time_ns is not None:
        print(f"HW exec time: {res.exec_time_ns} ns")
    return out



# revision 2
# speedup vs baseline: 3.8050x; 3.8050x over previous
"""Trainium2 Bass kernel for CustomGRUModel.

Reference computation (per batch row):
    gx = x @ W                       # [T, 3H] input projections
    per step t:
        gh_zr = h @ U[:, :2H]
        z = sigmoid(gxz + ghz + bz)
        r = sigmoid(gxr + ghr + br)
        n = tanh(gxn + (r*h) @ U[:, 2H:] + bn)
        h = z*h + (1-z)*n
    y = h_last @ Wd + bd

Sharding: data-parallel over batch, 32 rows per core on 8 cores. Weights
replicated. No collectives.

Per-core design (v2, bf16):
  - All matmul operands in bf16 (U, W, x, h, rh streams). fp32 is lowered
    as TWO HW passes per matmul with a full-rate LDWEIGHTS each and no
    fast-weight-load; bf16 is one pass with 2x FWL. PSUM accumulation
    stays fp32. Verified by simulation: rel_fro ~5e-3 vs the 2e-2 gate.
  - Layout "features on partitions": hT [H=512, B=32] as one SBUF tile
    [128, (k=4)x(b=32)]. Recurrent matmuls keep U tiles [128,128]
    stationary, stream hT chunks (N=32); gate outputs land [3H, B] in
    PSUM so elementwise runs on full 128 partitions.
  - gx is computed in chunks of TC=4 steps directly INTO the recurrence
    PSUM banks: the gate matmuls then accumulate on top (start=False),
    so no per-step gx adds on DVE at all. PSUM layout per chunk:
    [128, (m=12)(t=TC)(b=32)] fp32 = 3 banks, double buffered (6 banks),
    + 1 bank for x-transpose staging.
    PSUM "pending zero" discipline: chronologically-first matmul into
    each bank uses start=True (marks whole 2KB bank pending-zero:
    first-touch overwrites, then accumulates), everything after uses
    start=False; the chronologically-last (final step's gate matmul)
    uses stop=True.
  - Per step: r-gate matmuls first (m=4..7 chunk-pipelined into per-chunk
    sigmoid + rh multiply), then z matmuls and n matmuls (k-major,
    consuming rh chunks as they appear). h update: h = z*h + (1-z)*n with
    zc=1-z computed as sigmoid(-gz) on ScalarE (scale=-1). The tail
    (tanh/update) overlaps the next chunk's gx precompute on TensorE.
"""

import os

import numpy as np

B, T, D, H = 256, 512, 256, 512
NCORES = 8
BL = B // NCORES  # 32 batch rows per core
TC = 4  # timestep chunk for the gx precompute (bounded by PSUM: 3 banks/chunk)
KH = H // 128  # 4 k-tiles over H
KD = D // 128  # 2 k-tiles over D
M3H = 3 * H // 128  # 12 m-tiles over 3H

_CACHE = {}


def _build(t_run, with_bias):
    from contextlib import ExitStack

    import concourse.bacc as bacc
    import concourse.bass as bass
    import concourse.tile as tile
    from concourse import masks, mybir

    dt = mybir.dt
    f32 = dt.float32
    bf16 = dt.bfloat16
    AF = mybir.ActivationFunctionType

    nchunk = t_run // TC

    nc = bacc.Bacc(
        "TRN2", target_bir_lowering=False, debug=False, num_devices=NCORES
    )
    x_d = nc.dram_tensor("x", [BL, T, D], f32, kind="ExternalInput")
    w_d = nc.dram_tensor("W", [D, 3 * H], f32, kind="ExternalInput")
    u_d = nc.dram_tensor("U", [H, 3 * H], f32, kind="ExternalInput")
    b_d = nc.dram_tensor("b", [3 * H], f32, kind="ExternalInput")
    wd_d = nc.dram_tensor("Wd", [H, 1], f32, kind="ExternalInput")
    bd_d = nc.dram_tensor("bd", [1], f32, kind="ExternalInput")
    y_d = nc.dram_tensor("y", [BL, 1], f32, kind="ExternalOutput")

    # chunked view of x: [chunk, tc, b, d]
    x_view = x_d.rearrange("b (c t) d -> c t b d", t=TC)

    with tile.TileContext(nc) as tc, ExitStack() as ctx:
        const = ctx.enter_context(tc.tile_pool(name="const", bufs=1))
        xin_pool = ctx.enter_context(tc.tile_pool(name="xin", bufs=4))
        xt_pool = ctx.enter_context(tc.tile_pool(name="xt", bufs=2))
        sb_pool = ctx.enter_context(tc.tile_pool(name="sb", bufs=3))
        gx_psum = ctx.enter_context(
            tc.tile_pool(name="gxp", bufs=2, space=bass.MemorySpace.PSUM)
        )
        xt_psum = ctx.enter_context(
            tc.tile_pool(name="xtp", bufs=2, space=bass.MemorySpace.PSUM)
        )

        # ---- constants (load fp32, cast to bf16 working copies) ----
        stage = const.tile([128, 3 * H], f32)
        u_sb = const.tile([128, KH, 3 * H], bf16)
        for k in range(KH):
            nc.sync.dma_start(stage[:], u_d[k * 128 : (k + 1) * 128, :])
            nc.vector.tensor_copy(u_sb[:, k, :], stage[:])
        w_sb = const.tile([128, KD, 3 * H], bf16)
        for k in range(KD):
            nc.sync.dma_start(stage[:], w_d[k * 128 : (k + 1) * 128, :])
            nc.vector.tensor_copy(w_sb[:, k, :], stage[:])

        b_sb = const.tile([128, M3H], f32)
        nc.sync.dma_start(b_sb[:], b_d.rearrange("(m p) -> p m", p=128))
        bneg_sb = const.tile([128, M3H], f32)
        nc.scalar.mul(bneg_sb[:], b_sb[:], -1.0)

        wd_stage = const.tile([128, KH], f32)
        nc.sync.dma_start(wd_stage[:], wd_d.rearrange("(k p) o -> p (k o)", p=128))
        wd_sb = const.tile([128, KH], bf16)
        nc.vector.tensor_copy(wd_sb[:], wd_stage[:])
        bd_sb = const.tile([1, 1], f32)
        nc.sync.dma_start(bd_sb[0:1, :], bd_d.rearrange("(o u) -> o u", u=1))
        bd_bf = const.tile([1, 1], bf16)
        nc.vector.tensor_copy(bd_bf[0:1, :], bd_sb[0:1, :])
        ident = const.tile([128, 128], f32)
        masks.make_identity(nc, ident[:])
        ones_sb = const.tile([1, BL], bf16)
        nc.gpsimd.memset(ones_sb[0:1, :], 1.0)

        # persistent hidden state hT: [128, (k, b)] = [128, 4*32], bf16
        h_sb = const.tile([128, KH * BL], bf16)
        nc.gpsimd.memset(h_sb[:], 0.0)

        warm_ps = xt_psum.tile([128, KD, 128], f32, name="warm", tag="xtp")
        nc.tensor.transpose(warm_ps[:, 0, :], ident[:], ident[:])

        gx_tiles = {}

        def make_units(c):
            """Emit-thunks for precomputing gx chunk c (TC steps) into PSUM.

            gx tile layout: [128, (m=12)(t=TC)(b=32)] fp32 = 3 PSUM banks;
            bank j holds m-tiles 4j..4j+3. First matmul into each bank
            (m = 0/4/8, kd=0) uses start=True; all others start=False.
            """
            gx_t = gx_psum.tile([128, M3H, TC, BL], f32, name="gx", tag="gxp")
            gx_tiles[c] = gx_t
            xin = xin_pool.tile([128, D], f32, name="xin", tag="xin")
            xt_ps = xt_psum.tile([128, KD, TC * BL], f32, name="xtp", tag="xtp")
            xt_sb = xt_pool.tile([128, KD, TC * BL], bf16, name="xt", tag="xt")
            units = []

            def load():
                nc.sync.dma_start(xin[:], x_view[c])

            def tr(kd):
                nc.tensor.transpose(
                    xt_ps[:, kd, :], xin[:, 128 * kd : 128 * (kd + 1)], ident[:]
                )

            def evict(kd):
                nc.scalar.copy(xt_sb[:, kd, :], xt_ps[:, kd, :])

            def mm(m):
                for kd in range(KD):
                    nc.tensor.matmul(
                        gx_t[:, m, :, :],
                        w_sb[:, kd, m * 128 : (m + 1) * 128],
                        xt_sb[:, kd, :],
                        start=(kd == 0 and m % 4 == 0),
                        stop=False,
                        skip_group_check=True,
                    )

            units.append(load)
            for kd in range(KD):
                units.append(lambda kd=kd: tr(kd))
            for kd in range(KD):
                units.append(lambda kd=kd: evict(kd))
            for m in range(M3H):
                units.append(lambda m=m: mm(m))
            return units

        def emit_step(c, j):
            """One GRU step; gates accumulate into gx chunk tile at t=j."""
            gx_t = gx_tiles[c]
            last = j == TC - 1  # last step of chunk: emit stop=True per bank

            r_sb = sb_pool.tile([128, KH * BL], bf16, name="r", tag="r")
            rh_sb = sb_pool.tile([128, KH * BL], bf16, name="rh", tag="rh")
            # r gates (m=4..7) chunk-pipelined: after each m-tile finishes,
            # sigmoid + r*h for that 32-col chunk so n-matmuls start early.
            for i, m in enumerate((4, 5, 6, 7)):
                for k in range(KH):
                    nc.tensor.matmul(
                        gx_t[:, m, j, :],
                        u_sb[:, k, m * 128 : (m + 1) * 128],
                        h_sb[:, k * BL : (k + 1) * BL],
                        start=False,
                        stop=(last and m == 7 and k == KH - 1),
                        skip_group_check=True,
                    )
                if with_bias:
                    nc.scalar.activation(
                        r_sb[:, i * BL : (i + 1) * BL],
                        gx_t[:, m, j, :],
                        AF.Sigmoid,
                        bias=b_sb[:, m : m + 1],
                    )
                else:
                    nc.scalar.activation(
                        r_sb[:, i * BL : (i + 1) * BL], gx_t[:, m, j, :], AF.Sigmoid
                    )
                nc.vector.tensor_mul(
                    rh_sb[:, i * BL : (i + 1) * BL],
                    r_sb[:, i * BL : (i + 1) * BL],
                    h_sb[:, i * BL : (i + 1) * BL],
                )

            # z gates (m=0..3)
            for m in range(4):
                for k in range(KH):
                    nc.tensor.matmul(
                        gx_t[:, m, j, :],
                        u_sb[:, k, m * 128 : (m + 1) * 128],
                        h_sb[:, k * BL : (k + 1) * BL],
                        start=False,
                        stop=(last and m == 3 and k == KH - 1),
                        skip_group_check=True,
                    )

            # n gates (m=8..11), k-major so rh chunks are consumed as ready
            for k in range(KH):
                for m in range(8, 12):
                    nc.tensor.matmul(
                        gx_t[:, m, j, :],
                        u_sb[:, k, m * 128 : (m + 1) * 128],
                        rh_sb[:, k * BL : (k + 1) * BL],
                        start=False,
                        stop=(last and m == 11 and k == KH - 1),
                        skip_group_check=True,
                    )

            z_sb = sb_pool.tile([128, KH * BL], bf16, name="z", tag="z")
            zc_sb = sb_pool.tile([128, KH * BL], bf16, name="zc", tag="zc")
            zh_sb = sb_pool.tile([128, KH * BL], bf16, name="zh", tag="zh")
            n_sb = sb_pool.tile([128, KH * BL], bf16, name="n", tag="n")
            if with_bias:
                for i in range(4):
                    nc.scalar.activation(
                        z_sb[:, i * BL : (i + 1) * BL],
                        gx_t[:, i, j, :],
                        AF.Sigmoid,
                        bias=b_sb[:, i : i + 1],
                    )
                for i in range(4):
                    nc.scalar.activation(
                        zc_sb[:, i * BL : (i + 1) * BL],
                        gx_t[:, i, j, :],
                        AF.Sigmoid,
                        bias=bneg_sb[:, i : i + 1],
                        scale=-1.0,
                    )
            else:
                nc.scalar.activation(
                    z_sb[:].rearrange("p (m b) -> p m b", m=4),
                    gx_t[:, 0:4, j, :],
                    AF.Sigmoid,
                )
                nc.scalar.activation(
                    zc_sb[:].rearrange("p (m b) -> p m b", m=4),
                    gx_t[:, 0:4, j, :],
                    AF.Sigmoid,
                    scale=-1.0,
                )
            nc.vector.tensor_mul(zh_sb[:], z_sb[:], h_sb[:])
            if with_bias:
                for i in range(4):
                    nc.scalar.activation(
                        n_sb[:, i * BL : (i + 1) * BL],
                        gx_t[:, 8 + i, j, :],
                        AF.Tanh,
                        bias=b_sb[:, 8 + i : 9 + i],
                    )
            else:
                nc.scalar.activation(
                    n_sb[:].rearrange("p (m b) -> p m b", m=4),
                    gx_t[:, 8:12, j, :],
                    AF.Tanh,
                )
            # h = z*h + (1-z)*n
            zcn_sb = sb_pool.tile([128, KH * BL], bf16, name="zcn", tag="zcn")
            nc.vector.tensor_mul(zcn_sb[:], zc_sb[:], n_sb[:])
            nc.vector.tensor_add(h_sb[:], zh_sb[:], zcn_sb[:])

        # ---- main emission ----
        # Chunk 0's precompute up front; chunk c+1's precompute interleaved
        # between chunk c's steps so it fills TensorE tail gaps.
        for u in make_units(0):
            u()
        for c in range(nchunk):
            pend = make_units(c + 1) if c + 1 < nchunk else []
            done = 0
            for j in range(TC):
                emit_step(c, j)
                want = (len(pend) * (j + 1) + TC - 1) // TC
                while done < min(want, len(pend)):
                    pend[done]()
                    done += 1
            while done < len(pend):
                pend[done]()
                done += 1

        # final dense head: y = h @ Wd + bd
        out_ps = xt_psum.tile([128, KD, 128], f32, name="outp", tag="xtp")
        for k in range(KH):
            nc.tensor.matmul(
                out_ps[0:BL, 0, 0:1],
                h_sb[:, k * BL : (k + 1) * BL],
                wd_sb[:, k : k + 1],
                start=(k == 0),
                stop=False,
            )
        nc.tensor.matmul(
            out_ps[0:BL, 0, 0:1],
            ones_sb[0:1, :],
            bd_bf[0:1, :],
            start=False,
            stop=True,
        )
        y_sb = sb_pool.tile([BL, 1], f32, name="y", tag="y")
        nc.vector.tensor_copy(y_sb[:], out_ps[0:BL, 0, 0:1])
        nc.sync.dma_start(y_d[:], y_sb[:])

    nc.compile()
    return nc


def kernel(x, W, U, b, Wd, bd):
    from concourse.bass_utils import run_bass_kernel_spmd

    t_run = int(os.environ.get("GRU_T_RUN", T))

    x = np.ascontiguousarray(np.asarray(x, dtype=np.float32))
    W = np.ascontiguousarray(np.asarray(W, dtype=np.float32))
    U = np.ascontiguousarray(np.asarray(U, dtype=np.float32))
    b = np.ascontiguousarray(np.asarray(b, dtype=np.float32))
    Wd = np.ascontiguousarray(np.asarray(Wd, dtype=np.float32))
    bd = np.ascontiguousarray(np.asarray(bd, dtype=np.float32))

    with_bias = bool(np.any(b != 0.0))
    key = (t_run, with_bias)
    if key not in _CACHE:
        _CACHE[key] = _build(t_run, with_bias)
    nc = _CACHE[key]

    in_maps = [
        {
            "x": np.ascontiguousarray(x[i * BL : (i + 1) * BL]),
            "W": W,
            "U": U,
            "b": b,
            "Wd": Wd,
            "bd": bd,
        }
        for i in range(NCORES)
    ]
    res = run_bass_kernel_spmd(
        nc,
        in_maps,
        core_ids=list(range(NCORES)),
        trace=os.environ.get("GRU_TRACE", "0") == "1",
    )
    out = np.concatenate([r["y"] for r in res.results], axis=0)
    if res.exec_time_ns is not None:
        print(f"HW exec time: {res.exec_time_ns} ns")
    return out


# revision 3
# speedup vs baseline: 4.7784x; 1.2558x over previous
"""Trainium2 Bass kernel for CustomGRUModel.

Reference computation (per batch row):
    gx = x @ W                       # [T, 3H] input projections
    per step t:
        gh_zr = h @ U[:, :2H]
        z = sigmoid(gxz + ghz + bz)
        r = sigmoid(gxr + ghr + br)
        n = tanh(gxn + (r*h) @ U[:, 2H:] + bn)
        h = z*h + (1-z)*n
    y = h_last @ Wd + bd

Sharding: data-parallel over batch, 32 rows per core on 8 cores. Weights
replicated. No collectives.

Per-core design (v2, bf16):
  - All matmul operands in bf16 (U, W, x, h, rh streams). fp32 is lowered
    as TWO HW passes per matmul with a full-rate LDWEIGHTS each and no
    fast-weight-load; bf16 is one pass with 2x FWL. PSUM accumulation
    stays fp32. Verified by simulation: rel_fro ~5e-3 vs the 2e-2 gate.
  - Layout "features on partitions": hT [H=512, B=32] as one SBUF tile
    [128, (k=4)x(b=32)]. Recurrent matmuls keep U tiles [128,128]
    stationary, stream hT chunks (N=32); gate outputs land [3H, B] in
    PSUM so elementwise runs on full 128 partitions.
  - gx is computed in chunks of TC=4 steps directly INTO the recurrence
    PSUM banks: the gate matmuls then accumulate on top (start=False),
    so no per-step gx adds on DVE at all. PSUM layout per chunk:
    [128, (m=12)(t=TC)(b=32)] fp32 = 3 banks, double buffered (6 banks),
    + 1 bank for x-transpose staging.
    PSUM "pending zero" discipline: chronologically-first matmul into
    each bank uses start=True (marks whole 2KB bank pending-zero:
    first-touch overwrites, then accumulates), everything after uses
    start=False; the chronologically-last (final step's gate matmul)
    uses stop=True.
  - Per step: r-gate matmuls first (m=4..7 chunk-pipelined into per-chunk
    sigmoid + rh multiply), then z matmuls and n matmuls (k-major,
    consuming rh chunks as they appear). h update: h = z*h + (1-z)*n with
    zc=1-z computed as sigmoid(-gz) on ScalarE (scale=-1). The tail
    (tanh/update) overlaps the next chunk's gx precompute on TensorE.
"""

import os

import numpy as np

B, T, D, H = 256, 512, 256, 512
NCORES = 8
BL = B // NCORES  # 32 batch rows per core
TC = 4  # timestep chunk for the gx precompute (bounded by PSUM: 3 banks/chunk)
KH = H // 128  # 4 k-tiles over H
KD = D // 128  # 2 k-tiles over D
M3H = 3 * H // 128  # 12 m-tiles over 3H

_CACHE = {}


def _build(t_run, with_bias):
    from contextlib import ExitStack

    import concourse.bacc as bacc
    import concourse.bass as bass
    import concourse.tile as tile
    from concourse import masks, mybir

    dt = mybir.dt
    f32 = dt.float32
    bf16 = dt.bfloat16
    AF = mybir.ActivationFunctionType

    nchunk = t_run // TC

    nc = bacc.Bacc(
        "TRN2", target_bir_lowering=False, debug=False, num_devices=NCORES
    )
    x_d = nc.dram_tensor("x", [BL, T, D], f32, kind="ExternalInput")
    w_d = nc.dram_tensor("W", [D, 3 * H], f32, kind="ExternalInput")
    u_d = nc.dram_tensor("U", [H, 3 * H], f32, kind="ExternalInput")
    b_d = nc.dram_tensor("b", [3 * H], f32, kind="ExternalInput")
    wd_d = nc.dram_tensor("Wd", [H, 1], f32, kind="ExternalInput")
    bd_d = nc.dram_tensor("bd", [1], f32, kind="ExternalInput")
    y_d = nc.dram_tensor("y", [BL, 1], f32, kind="ExternalOutput")

    # chunked view of x: [chunk, tc, b, d]
    x_view = x_d.rearrange("b (c t) d -> c t b d", t=TC)

    with tile.TileContext(nc) as tc, ExitStack() as ctx:
        const = ctx.enter_context(tc.tile_pool(name="const", bufs=1))
        xin_pool = ctx.enter_context(tc.tile_pool(name="xin", bufs=4))
        xt_pool = ctx.enter_context(tc.tile_pool(name="xt", bufs=2))
        sb_pool = ctx.enter_context(tc.tile_pool(name="sb", bufs=3))
        gx_psum = ctx.enter_context(
            tc.tile_pool(name="gxp", bufs=2, space=bass.MemorySpace.PSUM)
        )
        xt_psum = ctx.enter_context(
            tc.tile_pool(name="xtp", bufs=2, space=bass.MemorySpace.PSUM)
        )

        # ---- constants (load fp32, cast to bf16 working copies) ----
        stage = const.tile([128, 3 * H], f32)
        u_sb = const.tile([128, KH, 3 * H], bf16)
        for k in range(KH):
            nc.sync.dma_start(stage[:], u_d[k * 128 : (k + 1) * 128, :])
            nc.vector.tensor_copy(u_sb[:, k, :], stage[:])
        w_sb = const.tile([128, KD, 3 * H], bf16)
        for k in range(KD):
            nc.sync.dma_start(stage[:], w_d[k * 128 : (k + 1) * 128, :])
            nc.vector.tensor_copy(w_sb[:, k, :], stage[:])

        b_sb = const.tile([128, M3H], f32)
        nc.sync.dma_start(b_sb[:], b_d.rearrange("(m p) -> p m", p=128))
        bneg_sb = const.tile([128, M3H], f32)
        nc.scalar.mul(bneg_sb[:], b_sb[:], -1.0)

        wd_stage = const.tile([128, KH], f32)
        nc.sync.dma_start(wd_stage[:], wd_d.rearrange("(k p) o -> p (k o)", p=128))
        wd_sb = const.tile([128, KH], bf16)
        nc.vector.tensor_copy(wd_sb[:], wd_stage[:])
        bd_sb = const.tile([1, 1], f32)
        nc.sync.dma_start(bd_sb[0:1, :], bd_d.rearrange("(o u) -> o u", u=1))
        bd_bf = const.tile([1, 1], bf16)
        nc.vector.tensor_copy(bd_bf[0:1, :], bd_sb[0:1, :])
        ident = const.tile([128, 128], f32)
        masks.make_identity(nc, ident[:])
        ones_sb = const.tile([1, BL], bf16)
        nc.gpsimd.memset(ones_sb[0:1, :], 1.0)

        # persistent hidden state hT: [128, (k, b)] = [128, 4*32], bf16
        h_sb = const.tile([128, KH * BL], bf16)
        nc.gpsimd.memset(h_sb[:], 0.0)

        warm_ps = xt_psum.tile([128, KD, 128], f32, name="warm", tag="xtp")
        nc.tensor.transpose(warm_ps[:, 0, :], ident[:], ident[:])

        gx_tiles = {}

        def make_units(c):
            """Emit-thunks for precomputing gx chunk c (TC steps) into PSUM.

            gx tile layout: [128, (m=12)(t=TC)(b=32)] fp32 = 3 PSUM banks;
            bank j holds m-tiles 4j..4j+3. First matmul into each bank
            (m = 0/4/8, kd=0) uses start=True; all others start=False.
            """
            gx_t = gx_psum.tile([128, M3H, TC, BL], f32, name="gx", tag="gxp")
            gx_tiles[c] = gx_t
            xin = xin_pool.tile([128, D], f32, name="xin", tag="xin")
            xt_ps = xt_psum.tile([128, KD, TC * BL], f32, name="xtp", tag="xtp")
            xt_sb = xt_pool.tile([128, KD, TC * BL], bf16, name="xt", tag="xt")
            units = []

            def load():
                nc.sync.dma_start(xin[:], x_view[c])

            def tr(kd):
                nc.tensor.transpose(
                    xt_ps[:, kd, :], xin[:, 128 * kd : 128 * (kd + 1)], ident[:]
                )

            def evict(kd):
                nc.scalar.copy(xt_sb[:, kd, :], xt_ps[:, kd, :])

            def mm(m):
                for kd in range(KD):
                    nc.tensor.matmul(
                        gx_t[:, m, :, :],
                        w_sb[:, kd, m * 128 : (m + 1) * 128],
                        xt_sb[:, kd, :],
                        start=(kd == 0 and m % 4 == 0),
                        stop=False,
                        skip_group_check=True,
                    )

            units.append(load)
            for kd in range(KD):
                units.append(lambda kd=kd: tr(kd))
            for kd in range(KD):
                units.append(lambda kd=kd: evict(kd))
            for m in range(M3H):
                units.append(lambda m=m: mm(m))
            return units

        def emit_step(c, j, mid_units):
            """One GRU step; gates accumulate into gx chunk tile at t=j.

            mid_units: precompute emit-thunks to splice in between the zr
            and n matmul blocks (fills the TensorE wait on rh without
            FIFO-blocking behind the n matmuls).
            """
            gx_t = gx_tiles[c]
            last = j == TC - 1  # last step of chunk: emit stop=True per bank

            # r gates (m=4..7) first, then z (m=0..3): all only need h.
            for m in (4, 5, 6, 7, 0, 1, 2, 3):
                for k in range(KH):
                    nc.tensor.matmul(
                        gx_t[:, m, j, :],
                        u_sb[:, k, m * 128 : (m + 1) * 128],
                        h_sb[:, k * BL : (k + 1) * BL],
                        start=False,
                        stop=(last and m == 3 and k == KH - 1),
                        skip_group_check=True,
                    )

            r_sb = sb_pool.tile([128, KH * BL], bf16, name="r", tag="r")
            rh_sb = sb_pool.tile([128, KH * BL], bf16, name="rh", tag="rh")
            z_sb = sb_pool.tile([128, KH * BL], bf16, name="z", tag="z")
            zc_sb = sb_pool.tile([128, KH * BL], bf16, name="zc", tag="zc")
            zh_sb = sb_pool.tile([128, KH * BL], bf16, name="zh", tag="zh")
            n_sb = sb_pool.tile([128, KH * BL], bf16, name="n", tag="n")
            zcn_sb = sb_pool.tile([128, KH * BL], bf16, name="zcn", tag="zcn")

            if with_bias:
                for i in range(4):
                    nc.scalar.activation(
                        r_sb[:, i * BL : (i + 1) * BL],
                        gx_t[:, 4 + i, j, :],
                        AF.Sigmoid,
                        bias=b_sb[:, 4 + i : 5 + i],
                    )
                for i in range(4):
                    nc.scalar.activation(
                        z_sb[:, i * BL : (i + 1) * BL],
                        gx_t[:, i, j, :],
                        AF.Sigmoid,
                        bias=b_sb[:, i : i + 1],
                    )
            else:
                nc.scalar.activation(
                    r_sb[:].rearrange("p (m b) -> p m b", m=4),
                    gx_t[:, 4:8, j, :],
                    AF.Sigmoid,
                )
                nc.scalar.activation(
                    z_sb[:].rearrange("p (m b) -> p m b", m=4),
                    gx_t[:, 0:4, j, :],
                    AF.Sigmoid,
                )
            nc.vector.tensor_mul(rh_sb[:], r_sb[:], h_sb[:])
            nc.vector.tensor_scalar(
                zc_sb[:], z_sb[:], -1.0, 1.0,
                mybir.AluOpType.mult, mybir.AluOpType.add,
            )
            nc.vector.tensor_mul(zh_sb[:], z_sb[:], h_sb[:])

            # precompute filler while TensorE would wait on rh
            for u in mid_units:
                u()

            # n gates (m=8..11)
            for k in range(KH):
                for m in range(8, 12):
                    nc.tensor.matmul(
                        gx_t[:, m, j, :],
                        u_sb[:, k, m * 128 : (m + 1) * 128],
                        rh_sb[:, k * BL : (k + 1) * BL],
                        start=False,
                        stop=(last and m == 11 and k == KH - 1),
                        skip_group_check=True,
                    )

            if with_bias:
                for i in range(4):
                    nc.scalar.activation(
                        n_sb[:, i * BL : (i + 1) * BL],
                        gx_t[:, 8 + i, j, :],
                        AF.Tanh,
                        bias=b_sb[:, 8 + i : 9 + i],
                    )
            else:
                nc.scalar.activation(
                    n_sb[:].rearrange("p (m b) -> p m b", m=4),
                    gx_t[:, 8:12, j, :],
                    AF.Tanh,
                )
            # h = z*h + (1-z)*n
            nc.vector.tensor_mul(zcn_sb[:], zc_sb[:], n_sb[:])
            nc.vector.tensor_add(h_sb[:], zh_sb[:], zcn_sb[:])

        # ---- main emission ----
        # Chunk 0's precompute up front; chunk c+1's precompute interleaved
        # into chunk c's steps (half mid-step, half at step boundary) so it
        # fills TensorE gaps while it waits on rh / the h update.
        for u in make_units(0):
            u()
        for c in range(nchunk):
            pend = make_units(c + 1) if c + 1 < nchunk else []
            done = 0
            for j in range(TC):
                want_mid = (len(pend) * (2 * j + 1) + 2 * TC - 1) // (2 * TC)
                mid = []
                while done < min(want_mid, len(pend)):
                    mid.append(pend[done])
                    done += 1
                emit_step(c, j, mid)
                want = (len(pend) * (j + 1) + TC - 1) // TC
                while done < min(want, len(pend)):
                    pend[done]()
                    done += 1
            while done < len(pend):
                pend[done]()
                done += 1

        # final dense head: y = h @ Wd + bd
        out_ps = xt_psum.tile([128, KD, 128], f32, name="outp", tag="xtp")
        for k in range(KH):
            nc.tensor.matmul(
                out_ps[0:BL, 0, 0:1],
                h_sb[:, k * BL : (k + 1) * BL],
                wd_sb[:, k : k + 1],
                start=(k == 0),
                stop=False,
            )
        nc.tensor.matmul(
            out_ps[0:BL, 0, 0:1],
            ones_sb[0:1, :],
            bd_bf[0:1, :],
            start=False,
            stop=True,
        )
        y_sb = sb_pool.tile([BL, 1], f32, name="y", tag="y")
        nc.vector.tensor_copy(y_sb[:], out_ps[0:BL, 0, 0:1])
        nc.sync.dma_start(y_d[:], y_sb[:])

    nc.compile()
    return nc


def kernel(x, W, U, b, Wd, bd):
    from concourse.bass_utils import run_bass_kernel_spmd

    t_run = int(os.environ.get("GRU_T_RUN", T))

    x = np.ascontiguousarray(np.asarray(x, dtype=np.float32))
    W = np.ascontiguousarray(np.asarray(W, dtype=np.float32))
    U = np.ascontiguousarray(np.asarray(U, dtype=np.float32))
    b = np.ascontiguousarray(np.asarray(b, dtype=np.float32))
    Wd = np.ascontiguousarray(np.asarray(Wd, dtype=np.float32))
    bd = np.ascontiguousarray(np.asarray(bd, dtype=np.float32))

    with_bias = bool(np.any(b != 0.0))
    key = (t_run, with_bias)
    if key not in _CACHE:
        _CACHE[key] = _build(t_run, with_bias)
    nc = _CACHE[key]

    in_maps = [
        {
            "x": np.ascontiguousarray(x[i * BL : (i + 1) * BL]),
            "W": W,
            "U": U,
            "b": b,
            "Wd": Wd,
            "bd": bd,
        }
        for i in range(NCORES)
    ]
    res = run_bass_kernel_spmd(
        nc,
        in_maps,
        core_ids=list(range(NCORES)),
        trace=os.environ.get("GRU_TRACE", "0") == "1",
    )
    out = np.concatenate([r["y"] for r in res.results], axis=0)
    if res.exec_time_ns is not None:
        print(f"HW exec time: {res.exec_time_ns} ns")
    return out


# revision 9
# speedup vs baseline: 6.8062x; 1.4244x over previous
"""Trainium2 Bass kernel for CustomGRUModel.

Reference computation (per batch row):
    gx = x @ W                       # [T, 3H] input projections
    per step t:
        gh_zr = h @ U[:, :2H]
        z = sigmoid(gxz + ghz + bz)
        r = sigmoid(gxr + ghr + br)
        n = tanh(gxn + (r*h) @ U[:, 2H:] + bn)
        h = z*h + (1-z)*n
    y = h_last @ Wd + bd

Sharding: data-parallel over batch, 32 rows per core on 8 cores. Weights
replicated. No collectives.

Per-core design (v2, bf16):
  - All matmul operands in bf16 (U, W, x, h, rh streams). fp32 is lowered
    as TWO HW passes per matmul with a full-rate LDWEIGHTS each and no
    fast-weight-load; bf16 is one pass with 2x FWL. PSUM accumulation
    stays fp32. Verified by simulation: rel_fro ~5e-3 vs the 2e-2 gate.
  - Layout "features on partitions": hT [H=512, B=32] as one SBUF tile
    [128, (k=4)x(b=32)]. Recurrent matmuls keep U tiles [128,128]
    stationary, stream hT chunks (N=32); gate outputs land [3H, B] in
    PSUM so elementwise runs on full 128 partitions.
  - gx is computed in chunks of TC=4 steps directly INTO the recurrence
    PSUM banks: the gate matmuls then accumulate on top (start=False),
    so no per-step gx adds on DVE at all. PSUM layout per chunk:
    [128, (m=12)(t=TC)(b=32)] fp32 = 3 banks, double buffered (6 banks),
    + 1 bank for x-transpose staging.
    PSUM "pending zero" discipline: chronologically-first matmul into
    each bank uses start=True (marks whole 2KB bank pending-zero:
    first-touch overwrites, then accumulates), everything after uses
    start=False; the chronologically-last (final step's gate matmul)
    uses stop=True.
  - Per step: r-gate matmuls first (m=4..7 chunk-pipelined into per-chunk
    sigmoid + rh multiply), then z matmuls and n matmuls (k-major,
    consuming rh chunks as they appear). h update: h = z*h + (1-z)*n with
    zc=1-z computed as sigmoid(-gz) on ScalarE (scale=-1). The tail
    (tanh/update) overlaps the next chunk's gx precompute on TensorE.
"""

import os

import numpy as np

B, T, D, H = 256, 512, 256, 512
NCORES = 8
BL = B // NCORES  # 32 batch rows per core
TC = 4  # timestep chunk for the gx precompute (bounded by PSUM: 3 banks/chunk)
KH = H // 128  # 4 k-tiles over H
KD = D // 128  # 2 k-tiles over D
M3H = 3 * H // 128  # 12 m-tiles over 3H

_CACHE = {}


def _build(t_run, with_bias):
    from contextlib import ExitStack

    import concourse.bacc as bacc
    import concourse.bass as bass
    import concourse.tile as tile
    from concourse import masks, mybir

    dt = mybir.dt
    f32 = dt.float32
    bf16 = dt.bfloat16
    AF = mybir.ActivationFunctionType

    nchunk = t_run // TC

    nc = bacc.Bacc(
        "TRN2", target_bir_lowering=False, debug=False, num_devices=NCORES
    )
    x_d = nc.dram_tensor("x", [BL, T, D], f32, kind="ExternalInput")
    w_d = nc.dram_tensor("W", [D, 3 * H], f32, kind="ExternalInput")
    u_d = nc.dram_tensor("U", [H, 3 * H], f32, kind="ExternalInput")
    b_d = nc.dram_tensor("b", [3 * H], f32, kind="ExternalInput")
    wd_d = nc.dram_tensor("Wd", [H, 1], f32, kind="ExternalInput")
    bd_d = nc.dram_tensor("bd", [1], f32, kind="ExternalInput")
    y_d = nc.dram_tensor("y", [BL, 1], f32, kind="ExternalOutput")

    # chunked view of x: [chunk, tc, b, d]
    x_view = x_d.rearrange("b (c t) d -> c t b d", t=TC)

    with tile.TileContext(nc) as tc, ExitStack() as ctx:
        const = ctx.enter_context(tc.tile_pool(name="const", bufs=1))
        xin_pool = ctx.enter_context(tc.tile_pool(name="xin", bufs=4))
        xt_pool = ctx.enter_context(tc.tile_pool(name="xt", bufs=2))
        sb_pool = ctx.enter_context(tc.tile_pool(name="sb", bufs=3))
        # one PSUM pool per gate group; each tile is exactly one 2KB bank
        # ([128, TC*4*BL] fp32 = 512 values) so Tile's dependency tracking
        # is per-gate and the bank pending-zero discipline is per-tile.
        gz_psum = ctx.enter_context(
            tc.tile_pool(name="gzp", bufs=2, space=bass.MemorySpace.PSUM)
        )
        gr_psum = ctx.enter_context(
            tc.tile_pool(name="grp", bufs=2, space=bass.MemorySpace.PSUM)
        )
        gn_psum = ctx.enter_context(
            tc.tile_pool(name="gnp", bufs=2, space=bass.MemorySpace.PSUM)
        )
        xt_psum = ctx.enter_context(
            tc.tile_pool(name="xtp", bufs=2, space=bass.MemorySpace.PSUM)
        )

        # ---- constants (load fp32, cast to bf16 working copies) ----
        stage = const.tile([128, 3 * H], f32)
        u_sb = const.tile([128, KH, 3 * H], bf16)
        for k in range(KH):
            nc.sync.dma_start(stage[:], u_d[k * 128 : (k + 1) * 128, :])
            nc.vector.tensor_copy(u_sb[:, k, :], stage[:])
        w_sb = const.tile([128, KD, 3 * H], bf16)
        for k in range(KD):
            nc.sync.dma_start(stage[:], w_d[k * 128 : (k + 1) * 128, :])
            nc.vector.tensor_copy(w_sb[:, k, :], stage[:])

        b_sb = const.tile([128, M3H], f32)
        nc.sync.dma_start(b_sb[:], b_d.rearrange("(m p) -> p m", p=128))
        bneg_sb = const.tile([128, M3H], f32)
        nc.scalar.mul(bneg_sb[:], b_sb[:], -1.0)

        wd_stage = const.tile([128, KH], f32)
        nc.sync.dma_start(wd_stage[:], wd_d.rearrange("(k p) o -> p (k o)", p=128))
        wd_sb = const.tile([128, KH], bf16)
        nc.vector.tensor_copy(wd_sb[:], wd_stage[:])
        bd_sb = const.tile([1, 1], f32)
        nc.sync.dma_start(bd_sb[0:1, :], bd_d.rearrange("(o u) -> o u", u=1))
        bd_bf = const.tile([1, 1], bf16)
        nc.vector.tensor_copy(bd_bf[0:1, :], bd_sb[0:1, :])
        ident = const.tile([128, 128], f32)
        masks.make_identity(nc, ident[:])
        ones_sb = const.tile([1, BL], bf16)
        nc.gpsimd.memset(ones_sb[0:1, :], 1.0)

        # persistent hidden state hT: [128, (k, b)] = [128, 4*32], bf16
        h_sb = const.tile([128, KH * BL], bf16)
        nc.gpsimd.memset(h_sb[:], 0.0)

        warm_ps = xt_psum.tile([128, KD, 128], f32, name="warm", tag="xtp")
        nc.tensor.transpose(warm_ps[:, 0, :], ident[:], ident[:])

        gx_tiles = {}

        def make_units(c):
            """Emit-thunks for precomputing gx chunk c (TC steps) into PSUM.

            Three per-gate tiles (z: m=0..3, r: 4..7, n: 8..11), layout
            [128, (t=TC)(mm=4)(b=32)] fp32 = one 2KB bank each, so the
            per-step gate slice [:, t, :, :] is 128 contiguous columns for
            the ScalarE reads. The first matmul into each tile (mm=0, kd=0)
            uses start=True; all later ones start=False.
            """
            parts = (
                gz_psum.tile([128, TC, 4, BL], f32, name="gz", tag="gzp"),
                gr_psum.tile([128, TC, 4, BL], f32, name="gr", tag="grp"),
                gn_psum.tile([128, TC, 4, BL], f32, name="gn", tag="gnp"),
            )
            gx_tiles[c] = parts
            xin = xin_pool.tile([128, D], f32, name="xin", tag="xin")
            xt_ps = xt_psum.tile([128, KD, TC * BL], f32, name="xtp", tag="xtp")
            xt_sb = xt_pool.tile([128, KD, TC * BL], bf16, name="xt", tag="xt")
            units = []

            def load():
                nc.sync.dma_start(xin[:], x_view[c])

            def tr(kd):
                nc.tensor.transpose(
                    xt_ps[:, kd, :], xin[:, 128 * kd : 128 * (kd + 1)], ident[:]
                )

            def evict(kd):
                nc.vector.tensor_copy(xt_sb[:, kd, :], xt_ps[:, kd, :])

            def mm(m):
                part = parts[m // 4]
                mm_i = m % 4
                for kd in range(KD):
                    nc.tensor.matmul(
                        part[:, :, mm_i, :],
                        w_sb[:, kd, m * 128 : (m + 1) * 128],
                        xt_sb[:, kd, :],
                        start=(kd == 0 and mm_i == 0),
                        stop=False,
                        skip_group_check=True,
                    )

            units.append(load)
            for kd in range(KD):
                units.append(lambda kd=kd: tr(kd))
            for kd in range(KD):
                units.append(lambda kd=kd: evict(kd))
            for m in range(M3H):
                units.append(lambda m=m: mm(m))
            return units

        def emit_step(c, j, mid_units):
            """One GRU step; gates accumulate into gx chunk tile at t=j.

            mid_units: precompute emit-thunks to splice in between the zr
            and n matmul blocks (fills the TensorE wait on rh without
            FIFO-blocking behind the n matmuls).
            """
            gz_t, gr_t, gn_t = gx_tiles[c]
            last = j == TC - 1  # last step of chunk: emit stop=True per bank

            # r gates (m=4..7) first, then z (m=0..3): all only need h.
            for m in (4, 5, 6, 7, 0, 1, 2, 3):
                part = (gz_t, gr_t)[m // 4]
                for k in range(KH):
                    nc.tensor.matmul(
                        part[:, j, m % 4, :],
                        u_sb[:, k, m * 128 : (m + 1) * 128],
                        h_sb[:, k * BL : (k + 1) * BL],
                        start=False,
                        stop=(last and m % 4 == 3 and k == KH - 1),
                        skip_group_check=True,
                    )

            r_sb = sb_pool.tile([128, KH * BL], bf16, name="r", tag="r")
            rh_sb = sb_pool.tile([128, KH * BL], bf16, name="rh", tag="rh")
            z_sb = sb_pool.tile([128, KH * BL], bf16, name="z", tag="z")
            zc_sb = sb_pool.tile([128, KH * BL], bf16, name="zc", tag="zc")
            zh_sb = sb_pool.tile([128, KH * BL], bf16, name="zh", tag="zh")
            n_sb = sb_pool.tile([128, KH * BL], bf16, name="n", tag="n")
            zcn_sb = sb_pool.tile([128, KH * BL], bf16, name="zcn", tag="zcn")

            if with_bias:
                for i in range(4):
                    nc.scalar.activation(
                        r_sb[:, i * BL : (i + 1) * BL],
                        gr_t[:, j, i, :],
                        AF.Sigmoid,
                        bias=b_sb[:, 4 + i : 5 + i],
                    )
                for i in range(4):
                    nc.scalar.activation(
                        z_sb[:, i * BL : (i + 1) * BL],
                        gz_t[:, j, i, :],
                        AF.Sigmoid,
                        bias=b_sb[:, i : i + 1],
                    )
            else:
                nc.scalar.activation(
                    r_sb[:].rearrange("p (m b) -> p m b", m=4),
                    gr_t[:, j, :, :],
                    AF.Sigmoid,
                )
                nc.scalar.activation(
                    z_sb[:].rearrange("p (m b) -> p m b", m=4),
                    gz_t[:, j, :, :],
                    AF.Sigmoid,
                )
            nc.vector.tensor_mul(rh_sb[:], r_sb[:], h_sb[:])
            nc.vector.tensor_scalar(
                zc_sb[:], z_sb[:], -1.0, 1.0,
                mybir.AluOpType.mult, mybir.AluOpType.add,
            )
            nc.vector.tensor_mul(zh_sb[:], z_sb[:], h_sb[:])

            # precompute filler while TensorE would wait on rh
            for u in mid_units:
                u()

            # n gates (m=8..11)
            for k in range(KH):
                for m in range(8, 12):
                    nc.tensor.matmul(
                        gn_t[:, j, m % 4, :],
                        u_sb[:, k, m * 128 : (m + 1) * 128],
                        rh_sb[:, k * BL : (k + 1) * BL],
                        start=False,
                        stop=(last and m == 11 and k == KH - 1),
                        skip_group_check=True,
                    )

            if with_bias:
                for i in range(4):
                    nc.scalar.activation(
                        n_sb[:, i * BL : (i + 1) * BL],
                        gn_t[:, j, i, :],
                        AF.Tanh,
                        bias=b_sb[:, 8 + i : 9 + i],
                    )
            else:
                nc.scalar.activation(
                    n_sb[:].rearrange("p (m b) -> p m b", m=4),
                    gn_t[:, j, :, :],
                    AF.Tanh,
                )
            # h = z*h + (1-z)*n
            nc.vector.tensor_mul(zcn_sb[:], zc_sb[:], n_sb[:])
            nc.vector.tensor_add(h_sb[:], zh_sb[:], zcn_sb[:])

        # ---- main emission ----
        # Chunk 0's precompute up front; chunk c+1's precompute interleaved
        # into chunk c's steps (half mid-step, half at step boundary) so it
        # fills TensorE gaps while it waits on rh / the h update.
        for u in make_units(0):
            u()
        for c in range(nchunk):
            pend = make_units(c + 1) if c + 1 < nchunk else []
            done = 0
            for j in range(TC):
                want_mid = (len(pend) * (2 * j + 1) + 2 * TC - 1) // (2 * TC)
                mid = []
                while done < min(want_mid, len(pend)):
                    mid.append(pend[done])
                    done += 1
                emit_step(c, j, mid)
                want = (len(pend) * (j + 1) + TC - 1) // TC
                while done < min(want, len(pend)):
                    pend[done]()
                    done += 1
            while done < len(pend):
                pend[done]()
                done += 1

        # final dense head: y = h @ Wd + bd
        out_ps = xt_psum.tile([128, KD, 128], f32, name="outp", tag="xtp")
        for k in range(KH):
            nc.tensor.matmul(
                out_ps[0:BL, 0, 0:1],
                h_sb[:, k * BL : (k + 1) * BL],
                wd_sb[:, k : k + 1],
                start=(k == 0),
                stop=False,
            )
        nc.tensor.matmul(
            out_ps[0:BL, 0, 0:1],
            ones_sb[0:1, :],
            bd_bf[0:1, :],
            start=False,
            stop=True,
        )
        y_sb = sb_pool.tile([BL, 1], f32, name="y", tag="y")
        nc.vector.tensor_copy(y_sb[:], out_ps[0:BL, 0, 0:1])
        nc.sync.dma_start(y_d[:], y_sb[:])

    nc.compile()
    return nc


def kernel(x, W, U, b, Wd, bd):
    from concourse.bass_utils import run_bass_kernel_spmd

    t_run = int(os.environ.get("GRU_T_RUN", T))

    x = np.ascontiguousarray(np.asarray(x, dtype=np.float32))
    W = np.ascontiguousarray(np.asarray(W, dtype=np.float32))
    U = np.ascontiguousarray(np.asarray(U, dtype=np.float32))
    b = np.ascontiguousarray(np.asarray(b, dtype=np.float32))
    Wd = np.ascontiguousarray(np.asarray(Wd, dtype=np.float32))
    bd = np.ascontiguousarray(np.asarray(bd, dtype=np.float32))

    with_bias = bool(np.any(b != 0.0))
    key = (t_run, with_bias)
    if key not in _CACHE:
        _CACHE[key] = _build(t_run, with_bias)
    nc = _CACHE[key]

    in_maps = [
        {
            "x": np.ascontiguousarray(x[i * BL : (i + 1) * BL]),
            "W": W,
            "U": U,
            "b": b,
            "Wd": Wd,
            "bd": bd,
        }
        for i in range(NCORES)
    ]
    res = run_bass_kernel_spmd(
        nc,
        in_maps,
        core_ids=list(range(NCORES)),
        trace=os.environ.get("GRU_TRACE", "0") == "1",
    )
    out = np.concatenate([r["y"] for r in res.results], axis=0)
    if res.exec_time_ns is not None:
        print(f"HW exec time: {res.exec_time_ns} ns")
    return out
